# revision 13
# baseline (speedup 1.0000x reference)
"""GCN message-passing kernel for Trainium2, 8 NeuronCores.

Net: 4x { h -> relu(segment_sum(h[src], dst) @ W + b) } (no relu on last).
N=100000 nodes, E=3200000 edges, dims 256->256->256->128->2.

Strategy (pull-model SpMM):
  - dst-nodes block-partitioned across 8 cores (12500/core, padded to 12544).
  - Node features live in a "table order": 7 chunks x (8 ranks x 1792 rows),
    so each chunked AllGather writes a contiguous table slice.
  - Per 128-dst block: gather h[src] rows via gpsimd.dma_gather (bf16 rows),
    build one-hot edge->slot indicators on DVE (is_equal vs iota), accumulate
    with PE matmuls into PSUM, then apply the layer linear + bias + relu.
  - Aggregation is linear, so layers 3/4 aggregate pre-transformed features:
    L3 aggregates t3 = h2 @ W3 (128-dim), L4 aggregates h3 (128-dim) and
    applies W4 after.  The same edge schedule drives all 4 layers.
  - int16 gather indices limit the table window to 32k rows, so the table is
    split in 4 quadrants; each block issues one dma_gather per quadrant.
"""
import sys
sys.path.insert(0, '/opt/trn_rl_repo')

import numpy as np
import ml_dtypes

from concourse import bass, mybir, tile, bacc
from concourse import bass_utils

dt = mybir.dt
bf16 = ml_dtypes.bfloat16

# ---------------- problem constants (hardcoded per spec) ----------------
N, E, D = 100000, 3200000, 256
OUTD = 2
N_CORES = 8
RPC = 12500                 # real nodes per core
NCHUNK = 7
BLK = 128
NBLK = 98
NLOC = NBLK * BLK           # 12544 padded rows per core
CHUNK = NLOC // NCHUNK      # 1792 rows per rank per chunk
BPC = NBLK // NCHUNK        # 14 blocks per chunk
NTAB = N_CORES * NLOC       # 100352 table rows
QROWS = NTAB // 4           # 25088 (< int16 max)
PAD_SLOT = 200.0


def _table_id(v):
    r = v // RPC
    l = v % RPC
    c = l // CHUNK
    p = l % CHUNK
    return c * (N_CORES * CHUNK) + r * CHUNK + p


def preprocess(src, dst):
    """Build per-core gather/slot streams and the shared segment-size tables."""
    tid_src = _table_id(src.astype(np.int64))
    core = dst // RPC
    dl = dst % RPC
    blk = dl // BLK
    slot = dl % BLK
    q = tid_src // QROWS
    qidx = (tid_src % QROWS).astype(np.int32)

    # group edges by (core, blk, q)
    per_core = []
    cnt = np.zeros((N_CORES, NBLK, 4), np.int64)
    order = np.lexsort((q, blk, core))
    core_s, blk_s, q_s = core[order], blk[order], q[order]
    qidx_s, slot_s = qidx[order], slot[order]
    np.add.at(cnt, (core_s, blk_s, q_s), 1)

    num_reg = cnt.max(axis=0)                         # [NBLK, 4]
    num_idx = ((num_reg + 127) // 128) * 128          # [NBLK, 4]
    nt_q = num_idx // 128
    nt_blk = nt_q.sum(axis=1)                         # tiles per block
    nt_max = int(nt_blk.max())
    tot_idx = int(num_idx.sum())
    pad_off = tot_idx                                  # all-zero fill segment
    tot_idx += nt_max * 128
    tot_tiles = int(nt_blk.sum())

    # stream offsets
    seg_off = np.zeros((NBLK, 4), np.int64)
    tile_off = np.zeros((NBLK, 4), np.int64)
    tb0 = np.zeros(NBLK, np.int64)                    # first global tile of blk
    o = t = 0
    for b in range(NBLK):
        tb0[b] = t
        for qq in range(4):
            seg_off[b, qq] = o
            tile_off[b, qq] = (t - tb0[b])            # tile offset inside block
            o += num_idx[b, qq]
            t += nt_q[b, qq]

    # per-core streams
    bounds = np.zeros((N_CORES, NBLK, 4, 2), np.int64)
    pos = 0
    for c in range(N_CORES):
        for b in range(NBLK):
            for qq in range(4):
                n = cnt[c, b, qq]
                bounds[c, b, qq] = (pos, pos + n)
                pos += n

    idx_streams, slot_streams = [], []
    for c in range(N_CORES):
        idx_s = np.zeros(tot_idx, np.int16)
        slt_s = np.full(tot_tiles * 128, PAD_SLOT, np.float32)
        for b in range(NBLK):
            for qq in range(4):
                lo, hi = bounds[c, b, qq]
                n = hi - lo
                o0 = seg_off[b, qq]
                idx_s[o0:o0 + n] = qidx_s[lo:hi]
                t0 = (tb0[b] + tile_off[b, qq]) * 128
                slt_s[t0:t0 + n] = slot_s[lo:hi]
        # wrap idx: position i -> [i % 16, i // 16], replicate to 128 partitions
        idx_w = idx_s.reshape(-1, 16).T
        idx_streams.append(np.tile(idx_w, (8, 1)))
        # slots: edge j of tile t -> [j, t]
        slot_streams.append(slt_s.reshape(tot_tiles, 128).T.astype(bf16))

    meta = dict(num_reg=num_reg, num_idx=num_idx, nt_q=nt_q, nt_blk=nt_blk,
                seg_off=seg_off, tile_off=tile_off, tb0=tb0,
                tot_idx=tot_idx, tot_tiles=tot_tiles, pad_off=pad_off,
                nt_max=nt_max)
    return meta, idx_streams, slot_streams


def build_program(meta, repeat=1, layers=4, skip_coll=False, skip_ind=False,
                  skip_gather=False):
    num_reg, num_idx = meta['num_reg'], meta['num_idx']
    nt_q, nt_blk, tb0 = meta['nt_q'], meta['nt_blk'], meta['tb0']
    seg_off, tile_off = meta['seg_off'], meta['tile_off']
    tot_idx, tot_tiles = meta['tot_idx'], meta['tot_tiles']
    NT_MAX = int(meta['nt_max'])
    PAD_OFF = int(meta['pad_off'])

    nc = bacc.Bacc("TRN2", target_bir_lowering=False, debug=False,
                   num_devices=N_CORES)

    F8 = dt.float8e4
    x_tab = nc.dram_tensor("x_tab", [NTAB, D], F8, kind="ExternalInput")
    idxs_d = nc.dram_tensor("idxs", [128, tot_idx // 16], dt.int16, kind="ExternalInput")
    slots_d = nc.dram_tensor("slots", [128, tot_tiles], dt.bfloat16, kind="ExternalInput")
    iota_d = nc.dram_tensor("iota", [128, 128], dt.bfloat16, kind="ExternalInput")
    ident_d = nc.dram_tensor("ident", [128, 128], dt.bfloat16, kind="ExternalInput")
    w1_d = nc.dram_tensor("w1", [256, 256], dt.bfloat16, kind="ExternalInput")
    w2_d = nc.dram_tensor("w2", [256, 256], dt.bfloat16, kind="ExternalInput")
    w3_d = nc.dram_tensor("w3", [256, 128], dt.bfloat16, kind="ExternalInput")
    w4_d = nc.dram_tensor("w4", [128, 2], dt.bfloat16, kind="ExternalInput")
    b1_d = nc.dram_tensor("b1", [1, 256], dt.bfloat16, kind="ExternalInput")
    b2_d = nc.dram_tensor("b2", [1, 256], dt.bfloat16, kind="ExternalInput")
    b3_d = nc.dram_tensor("b3", [1, 128], dt.bfloat16, kind="ExternalInput")
    b4_d = nc.dram_tensor("b4", [1, 2], dt.bfloat16, kind="ExternalInput")
    tok_d = nc.dram_tensor("tok", [128, 4], dt.float32, kind="ExternalInput")

    outp = nc.dram_tensor("outp", [2, NLOC], dt.float32, kind="ExternalOutput")
    tok_out = nc.dram_tensor("tok_out", [128, 4], dt.float32, kind="ExternalOutput")

    h1_tab = nc.dram_tensor("h1_tab", [NTAB, D], F8, addr_space="Shared")
    t3_tab = nc.dram_tensor("t3_tab", [NTAB, 128], dt.bfloat16, addr_space="Shared")
    h3_tab = nc.dram_tensor("h3_tab", [NTAB, 128], dt.bfloat16, addr_space="Shared")
    sh1 = [nc.dram_tensor(f"sh1_{c}", [CHUNK, D], F8) for c in range(NCHUNK)]
    st3 = [nc.dram_tensor(f"st3_{c}", [CHUNK, 128], dt.bfloat16) for c in range(NCHUNK)]
    sh3 = [nc.dram_tensor(f"sh3_{c}", [CHUNK, 128], dt.bfloat16) for c in range(NCHUNK)]

    RG = [list(range(N_CORES))]
    AF = mybir.ActivationFunctionType

    with tile.TileContext(nc) as tc:
        with tc.tile_pool(name="const", bufs=1) as cpool, \
             tc.tile_pool(name="msgp", bufs=4) as msgp, \
             tc.tile_pool(name="indp", bufs=3) as indp, \
             tc.tile_pool(name="work", bufs=3) as work, \
             tc.tile_pool(name="psA", bufs=3, space="PSUM") as psA, \
             tc.tile_pool(name="psB", bufs=2, space="PSUM") as psB, \
             tc.tile_pool(name="psT", bufs=2, space="PSUM") as psT, \
             tc.tile_pool(name="psO", bufs=1, space="PSUM") as psO:

            idx_all = cpool.tile([128, tot_idx // 16], dt.int16)
            nc.sync.dma_start(out=idx_all[:], in_=idxs_d[:])
            slot_all = cpool.tile([128, tot_tiles], dt.bfloat16)
            nc.sync.dma_start(out=slot_all[:], in_=slots_d[:])
            iota_t = cpool.tile([128, 128], dt.bfloat16)
            nc.sync.dma_start(out=iota_t[:], in_=iota_d[:])
            id_t = cpool.tile([128, 128], dt.bfloat16)
            nc.sync.dma_start(out=id_t[:], in_=ident_d[:])
            w1_t = cpool.tile([128, 2, 256], dt.bfloat16)
            nc.sync.dma_start(out=w1_t[:, 0, :], in_=w1_d[0:128, :])
            nc.sync.dma_start(out=w1_t[:, 1, :], in_=w1_d[128:256, :])
            w2_t = cpool.tile([128, 2, 256], dt.bfloat16)
            nc.sync.dma_start(out=w2_t[:, 0, :], in_=w2_d[0:128, :])
            nc.sync.dma_start(out=w2_t[:, 1, :], in_=w2_d[128:256, :])
            w3_t = cpool.tile([128, 2, 128], dt.bfloat16)
            nc.sync.dma_start(out=w3_t[:, 0, :], in_=w3_d[0:128, :])
            nc.sync.dma_start(out=w3_t[:, 1, :], in_=w3_d[128:256, :])
            w4_t = cpool.tile([128, 2], dt.bfloat16)
            nc.sync.dma_start(out=w4_t[:], in_=w4_d[:])
            b1_t = cpool.tile([1, 256], dt.bfloat16)
            nc.sync.dma_start(out=b1_t[:], in_=b1_d[:])
            b2_t = cpool.tile([1, 256], dt.bfloat16)
            nc.sync.dma_start(out=b2_t[:], in_=b2_d[:])
            b3_t = cpool.tile([1, 128], dt.bfloat16)
            nc.sync.dma_start(out=b3_t[:], in_=b3_d[:])
            b4_t = cpool.tile([1, 2], dt.bfloat16)
            nc.sync.dma_start(out=b4_t[:], in_=b4_d[:])
            ones_t = cpool.tile([1, 128], dt.bfloat16)
            nc.vector.memset(ones_t[:], 1.0)
            zrow_t = cpool.tile([1, 256], dt.bfloat16)
            nc.vector.memset(zrow_t[:], 0.0)
            tok_t = cpool.tile([128, 4], dt.float32)
            nc.sync.dma_start(out=tok_t[:], in_=tok_d[:])

            def gather_block(b, tab_ap, Dl, fill_slot=False, mdt=dt.bfloat16):
                ntb = int(nt_blk[b])
                nalloc = NT_MAX if fill_slot else ntb
                msgs = msgp.tile([128, nalloc, Dl], mdt, tag="msgs")
                if not skip_gather:
                    for qq in range(4):
                        ni = int(num_idx[b, qq])
                        if ni == 0:
                            continue
                        nr = ni
                        o16 = int(seg_off[b, qq]) // 16
                        to = int(tile_off[b, qq])
                        ntq = ni // 128
                        nc.gpsimd.dma_gather(
                            msgs[:, to:to + ntq, :],
                            tab_ap[qq * QROWS:(qq + 1) * QROWS, :],
                            idx_all[:, o16:o16 + ni // 16],
                            ni, nr, Dl, single_packet=False)
                    if fill_slot and ntb < NT_MAX:
                        ni = (NT_MAX - ntb) * 128
                        nc.gpsimd.dma_gather(
                            msgs[:, ntb:NT_MAX, :],
                            tab_ap[0:QROWS, :],
                            idx_all[:, PAD_OFF // 16:(PAD_OFF + ni) // 16],
                            ni, ni, Dl, single_packet=False)
                ind = indp.tile([128, ntb, 128], mdt, tag="ind")
                t0 = int(tb0[b])
                if not skip_ind:
                    nc.vector.tensor_tensor(
                        out=ind[:],
                        in0=slot_all[:, t0:t0 + ntb][:, :, None].to_broadcast([128, ntb, 128]),
                        in1=iota_t[:][:, None, :].to_broadcast([128, ntb, 128]),
                        op=mybir.AluOpType.is_equal)
                return msgs, ind, ntb

            def agg_block(b, tab_ap, Dl, transposed=False, close=True,
                          fill_slot=False, mdt=dt.bfloat16):
                if int(nt_blk[b]) == 0:
                    acc = psA.tile([128, Dl if not transposed else 128],
                                   dt.float32, space="PSUM", tag="acc")
                    nc.tensor.matmul(out=acc[:], lhsT=ones_t[:],
                                     rhs=zrow_t[:, :acc.shape[-1]],
                                     start=True, stop=close)
                    return acc
                msgs, ind, ntb = gather_block(b, tab_ap, Dl, fill_slot=fill_slot,
                                              mdt=mdt)
                if not transposed:
                    acc = psA.tile([128, Dl], dt.float32, space="PSUM", tag="acc")
                    for t in range(ntb):
                        nc.tensor.matmul(out=acc[:], lhsT=ind[:, t, :],
                                         rhs=msgs[:, t, :],
                                         start=(t == 0),
                                         stop=(close and t == ntb - 1))
                else:
                    acc = psA.tile([128, 128], dt.float32, space="PSUM", tag="acc")
                    for t in range(ntb):
                        nc.tensor.matmul(out=acc[:], lhsT=msgs[:, t, :],
                                         rhs=ind[:, t, :],
                                         start=(t == 0),
                                         stop=(close and t == ntb - 1))
                return acc

            def linear_from_agg(acc, w_t, b_t, nchunks, nout):
                """acc [128dst, 128*nchunks f] psum -> h psum [128dst, nout]."""
                aggb = work.tile([128, 128 * nchunks], dt.bfloat16, tag="aggb")
                nc.vector.tensor_copy(out=aggb[:], in_=acc[:])
                aggT = work.tile([128, 128 * nchunks], dt.bfloat16, tag="aggT")
                for k in range(nchunks):
                    tp = psT.tile([128, 128], dt.bfloat16, space="PSUM", tag="tp")
                    nc.tensor.transpose(out=tp[:], in_=aggb[:, k * 128:(k + 1) * 128],
                                        identity=id_t[:])
                    nc.vector.tensor_copy(out=aggT[:, k * 128:(k + 1) * 128], in_=tp[:])
                hps = psB.tile([128, nout], dt.float32, space="PSUM", tag="h")
                for k in range(nchunks):
                    nc.tensor.matmul(out=hps[:], lhsT=aggT[:, k * 128:(k + 1) * 128],
                                     rhs=w_t[:, k, :], start=(k == 0), stop=False)
                nc.tensor.matmul(out=hps[:], lhsT=ones_t[:], rhs=b_t[:],
                                 start=False, stop=True)
                return hps, aggT

            def maybe_ag(layer_i, b, shard, tab):
                if skip_coll:
                    return
                if b % BPC == BPC - 1:
                    c = b // BPC
                    nc.gpsimd.collective_compute(
                        "AllGather", mybir.AluOpType.bypass, replica_groups=RG,
                        ins=[shard[c][:]],
                        outs=[tab[c * N_CORES * CHUNK:(c + 1) * N_CORES * CHUNK, :]])

            for rep in range(repeat):
                # ---- layer 1: h1 = relu(agg(x) @ W1 + b1) ----
                for b in range(NBLK):
                    acc = agg_block(b, x_tab, D, mdt=F8)
                    hps, _ = linear_from_agg(acc, w1_t, b1_t, 2, 256)
                    h1_blk = work.tile([128, 256], F8, tag="hblk8")
                    nc.scalar.activation(out=h1_blk[:], in_=hps[:], func=AF.Relu)
                    r0 = (b % BPC) * BLK
                    nc.sync.dma_start(out=sh1[b // BPC][r0:r0 + BLK, :], in_=h1_blk[:])
                    maybe_ag(1, b, sh1, h1_tab)

                if layers < 2: continue
                # ---- layer 2: h2 = relu(agg(h1) @ W2 + b2); t3 = h2 @ W3 ----
                for b in range(NBLK):
                    acc = agg_block(b, h1_tab, D, mdt=F8)
                    hps, _ = linear_from_agg(acc, w2_t, b2_t, 2, 256)
                    h2_blk = work.tile([128, 256], dt.bfloat16, tag="hblk")
                    nc.scalar.activation(out=h2_blk[:], in_=hps[:], func=AF.Relu)
                    # t3 = h2 @ W3  (transpose h2, then 2 chunk matmuls)
                    h2T = work.tile([128, 256], dt.bfloat16, tag="h2T")
                    for k in range(2):
                        tp = psT.tile([128, 128], dt.bfloat16, space="PSUM", tag="tp")
                        nc.tensor.transpose(out=tp[:], in_=h2_blk[:, k * 128:(k + 1) * 128],
                                            identity=id_t[:])
                        nc.vector.tensor_copy(out=h2T[:, k * 128:(k + 1) * 128], in_=tp[:])
                    t3ps = psB.tile([128, 128], dt.float32, space="PSUM", tag="h")
                    for k in range(2):
                        nc.tensor.matmul(out=t3ps[:], lhsT=h2T[:, k * 128:(k + 1) * 128],
                                         rhs=w3_t[:, k, :], start=(k == 0), stop=(k == 1))
                    t3_blk = work.tile([128, 128], dt.bfloat16, tag="t3blk")
                    nc.vector.tensor_copy(out=t3_blk[:], in_=t3ps[:])
                    r0 = (b % BPC) * BLK
                    nc.sync.dma_start(out=st3[b // BPC][r0:r0 + BLK, :], in_=t3_blk[:])
                    maybe_ag(2, b, st3, t3_tab)

                if layers < 3: continue
                # ---- layer 3: h3 = relu(agg(t3) + b3) ----
                for b in range(NBLK):
                    acc = agg_block(b, t3_tab, 128, close=False)
                    nc.tensor.matmul(out=acc[:], lhsT=ones_t[:], rhs=b3_t[:],
                                     start=False, stop=True)
                    h3_blk = work.tile([128, 128], dt.bfloat16, tag="t3blk")
                    nc.scalar.activation(out=h3_blk[:], in_=acc[:], func=AF.Relu)
                    r0 = (b % BPC) * BLK
                    nc.sync.dma_start(out=sh3[b // BPC][r0:r0 + BLK, :], in_=h3_blk[:])
                    maybe_ag(3, b, sh3, h3_tab)

                if layers < 4: continue
                # ---- layer 4: out = agg(h3) @ W4 + b4 (transposed agg) ----
                for b in range(NBLK):
                    accT = agg_block(b, h3_tab, 128, transposed=True)
                    aT = work.tile([128, 128], dt.bfloat16, tag="t3blk")
                    nc.vector.tensor_copy(out=aT[:], in_=accT[:])
                    ops = psO.tile([2, 128], dt.float32, space="PSUM", tag="o")
                    nc.tensor.matmul(out=ops[:], lhsT=w4_t[:], rhs=aT[:],
                                     start=True, stop=False)
                    nc.tensor.matmul(out=ops[:], lhsT=b4_t[:], rhs=ones_t[:],
                                     start=False, stop=True)
                    ob = work.tile([2, 128], dt.float32, tag="ob")
                    nc.vector.tensor_copy(out=ob[:], in_=ops[:])
                    nc.sync.dma_start(out=outp[:, b * BLK:(b + 1) * BLK], in_=ob[:])

            # token passthrough (anti-CSE for timing harness)
            nc.scalar.mul(tok_t[:], tok_t[:], 2.0)
            nc.sync.dma_start(out=tok_out[:], in_=tok_t[:])

    nc.compile()
    return nc


def make_in_maps(x, src, dst, W1, b1, W2, b2, W3, b3, W4, b4,
                 meta, idx_streams, slot_streams):
    f8 = ml_dtypes.float8_e4m3
    v = np.arange(N)
    tid = _table_id(v)
    x_tab = np.zeros((NTAB, D), f8)
    x_tab[tid] = np.asarray(x).astype(f8)
    iota_np = np.tile(np.arange(128, dtype=np.float32)[None, :], (128, 1)).astype(bf16)
    ident_np = np.eye(128, dtype=np.float32).astype(bf16)
    common = {
        "x_tab": x_tab,
        "iota": iota_np, "ident": ident_np,
        "w1": np.asarray(W1).astype(bf16), "w2": np.asarray(W2).astype(bf16),
        "w3": np.asarray(W3).astype(bf16), "w4": np.asarray(W4).astype(bf16),
        "b1": np.asarray(b1).reshape(1, -1).astype(bf16),
        "b2": np.asarray(b2).reshape(1, -1).astype(bf16),
        "b3": np.asarray(b3).reshape(1, -1).astype(bf16),
        "b4": np.asarray(b4).reshape(1, -1).astype(bf16),
        "tok": np.zeros((128, 4), np.float32),
    }
    in_maps = []
    for c in range(N_CORES):
        m = dict(common)
        m["idxs"] = idx_streams[c]
        m["slots"] = slot_streams[c]
        in_maps.append(m)
    return in_maps


def assemble_output(results):
    out = np.zeros((N, OUTD), np.float32)
    for c in range(N_CORES):
        o = results[c]["outp"]            # [2, NLOC]
        out[c * RPC:(c + 1) * RPC, :] = o.T[:RPC, :]
    return out


_CACHE = {}
LAST = {}


def kernel(x, src, dst, W1, b1, W2, b2, W3, b3, W4, b4):
    src = np.asarray(src)
    dst = np.asarray(dst)
    key = (src.tobytes(), dst.tobytes())
    kh = hash(key)
    if kh in _CACHE:
        meta, idx_streams, slot_streams, nc = _CACHE[kh]
    else:
        meta, idx_streams, slot_streams = preprocess(src, dst)
        nc = build_program(meta)
        _CACHE[kh] = (meta, idx_streams, slot_streams, nc)
    in_maps = make_in_maps(x, src, dst, W1, b1, W2, b2, W3, b3, W4, b4,
                           meta, idx_streams, slot_streams)
    LAST.update(nc=nc, in_maps=in_maps, meta=meta)
    res = bass_utils.run_bass_kernel_spmd(nc, in_maps, core_ids=list(range(N_CORES)))
    return assemble_output(res.results)

